# revision 44
# baseline (speedup 1.0000x reference)
"""Trainium2 Bass kernel for nn_Encoders_13451837571792.

2-layer (shared-weight) transformer encoder, B=4 S=1024 DM=512 H=8 DFF=2048,
with a global 2D softmax over each (b,h) attention matrix and o = A^T @ v.

Sharding over 8 NeuronCores: core c owns (batch b=c//2, head-group g=c%2:
heads 4g..4g+3) for attention, and token block g of batch b for the
wo-projection / LayerNorms / FFN.  All cross-core exchange is PAIR-wise
(cores 2b and 2b+1 of the same batch): after attention each core ships its
heads' o restricted to the partner's token block, and after each non-final
layer it ships its updated token block h, via AllGather with
replica_groups=[[0,1],[2,3],[4,5],[6,7]].  Tokens are kept in CORE-RELATIVE
order ([own block, partner block]) so every local/remote split is a
compile-time slice; the host permutes x/mask/pad per core and permutes wo's
rows per core ([own heads' features, partner's]) to match.

All activations are kept feature-major ([feature-partition, token-free]) so
every matmul contraction sits on partitions.  The matmul datapath runs in
bfloat16 with fp32 PSUM accumulation (measured f32r runs at ~2.2 cycles/row
on TRN2 HW; bf16 runs at 1), while the residual/LayerNorm stream is kept in
f32r with bf16 shadow copies for matmul operands (bf16 residuals alone cost
~1e-2 rel err; split-stream costs ~2.5e-3).  Masking is folded into the
logits matmul as two extra contraction rows; softmax subtracts a fixed safe
bias EXPB (exp of masked entries underflows to exactly 0).  Z comes from a
ones-column appended to v in the o = E^T v matmul (row 64 of the PSUM
accumulator is sum_j E[j,i]), so the exp pass needs no accumulator reads.
The attention inner loop is software-pipelined (logits(jb+1) issues before
Ev(jb)) so the PE never stalls on the scalar engine's exp; the FFN loop is
pipelined the same way around the vector engine's relu.
"""

import numpy as np
import ml_dtypes

import concourse.bass as bass
import concourse.bacc as bacc
import concourse.tile as tile
import concourse.mybir as mybir
from concourse.bass_utils import run_bass_kernel_spmd

B, S, DM, H, DFF = 4, 1024, 512, 8, 2048
D, P, NC = 64, 128, 8
FS = DM // P          # 4 feature subtiles
DS2 = DFF // P        # 16 dff subtiles
TOK = S // 2          # 512 tokens per core
JBN = S // P          # 8 j-blocks
HPC = H // 2          # 4 heads per core
EXPB = 48.0           # fixed softmax bias (safe: |logits| << 48+87)
EPS = 1e-9
PAIRS = [[2 * i, 2 * i + 1] for i in range(4)]

f32 = mybir.dt.float32
f32r = mybir.dt.float32r
bf16 = mybir.dt.bfloat16
BF = ml_dtypes.bfloat16
FT = mybir.ActivationFunctionType
ALU = mybir.AluOpType


def _register_const_ap(nc, dtype, value):
    t = nc.alloc_sbuf_tensor(f"const-{dtype.name}-{value}", [128, 1], dtype)
    nc.gpsimd.memset(t.ap(), value)
    nc.const_aps.aps[(dtype, value)] = t.ap()
    nc.all_engine_barrier()


def build_program(layer_num: int, nz: float, structured: bool):
    # All ACT funcs used here (Exp, Ln, Identity, Square, Copy) live in the
    # natural_log_exp_and_others table set; restricting the selector to it
    # collapses ping-ponging ACT_TABLE_LOADs into one.
    if not getattr(bacc, "_ant_tables_patched", False):
        _orig_get_tables = bacc.get_activation_tables

        def _prefer_nle(arch):
            tabs = _orig_get_tables(arch)
            if "natural_log_exp_and_others" not in tabs:
                return tabs
            mine = {"Exp", "Ln", "Identity", "Square", "Copy"}
            out = {}
            for k, v in tabs.items():
                if k == "natural_log_exp_and_others":
                    out[k] = v
                else:
                    out[k] = {f for f in v if str(f).split(".")[-1] not in mine}
            return out

        bacc.get_activation_tables = _prefer_nle
        bacc._ant_tables_patched = True
    nc = bacc.Bacc("TRN2", target_bir_lowering=False, debug=False, num_devices=NC)
    _register_const_ap(nc, f32, -EXPB)
    _register_const_ap(nc, f32, EPS)

    # ---------------- DRAM I/O ----------------
    xT = nc.dram_tensor("xT", [P, FS, S], bf16, kind="ExternalInput")
    res0 = nc.dram_tensor("res0", [P, FS, TOK], f32r, kind="ExternalInput")
    qrow = nc.dram_tensor("qrow", [2, S], bf16, kind="ExternalInput")
    krow = nc.dram_tensor("krow", [2, S], bf16, kind="ExternalInput")
    if not structured:
        negm = nc.dram_tensor("negm", [P, JBN, S], f32, kind="ExternalInput")
    wq8 = nc.dram_tensor("wq8", [P, FS, 2 * P], bf16, kind="ExternalInput")
    wk = nc.dram_tensor("wk", [P, FS, 2 * P], bf16, kind="ExternalInput")
    wv = nc.dram_tensor("wv", [P, FS, 2 * P], bf16, kind="ExternalInput")
    wo = nc.dram_tensor("wo", [P, FS, DM], bf16, kind="ExternalInput")
    w1 = nc.dram_tensor("w1", [P, FS, DFF], bf16, kind="ExternalInput")
    w2 = nc.dram_tensor("w2", [P, DS2, DM], bf16, kind="ExternalInput")
    biasq = nc.dram_tensor("biasq", [P, 2], f32, kind="ExternalInput")
    biask = nc.dram_tensor("biask", [P, 2], f32, kind="ExternalInput")
    bvb = nc.dram_tensor("bvb", [P, 2 * P], f32, kind="ExternalInput")
    bo_g = nc.dram_tensor("bo_g", [P, FS], f32, kind="ExternalInput")
    b1_g = nc.dram_tensor("b1_g", [P, DS2], f32, kind="ExternalInput")
    b2_g = nc.dram_tensor("b2_g", [P, FS], f32, kind="ExternalInput")
    g1_g = nc.dram_tensor("g1_g", [P, FS], f32, kind="ExternalInput")
    be1_g = nc.dram_tensor("be1_g", [P, FS], f32, kind="ExternalInput")
    g2_g = nc.dram_tensor("g2_g", [P, FS], f32, kind="ExternalInput")
    be2_g = nc.dram_tensor("be2_g", [P, FS], f32, kind="ExternalInput")
    onesK1 = nc.dram_tensor("onesK1", [1, P], f32r, kind="ExternalInput")
    identd = nc.dram_tensor("identd", [P, P], f32r, kind="ExternalInput")
    rm128d = nc.dram_tensor("rm128d", [P, 1], f32r, kind="ExternalInput")
    borow_d = nc.dram_tensor("borow_d", [1, DM], bf16, kind="ExternalInput")
    b2row_d = nc.dram_tensor("b2row_d", [1, DM], bf16, kind="ExternalInput")
    onestok_d = nc.dram_tensor("onestok_d", [1, TOK], bf16, kind="ExternalInput")
    out = nc.dram_tensor("out", [TOK, DM], f32, kind="ExternalOutput")

    o_in = [[nc.dram_tensor(f"o_in_{l}_{pr}", [P, TOK], bf16) for pr in range(2)]
            for l in range(layer_num)]
    o_out = [[nc.dram_tensor(f"o_out_{l}_{pr}", [2, P, TOK], bf16)
              for pr in range(2)]
             for l in range(layer_num)]
    h_in = [nc.dram_tensor(f"h_in_{l}", [FS, P, TOK], bf16)
            for l in range(layer_num - 1)]
    h_out = [
        nc.dram_tensor(f"h_out_{l}", [2, FS, P, TOK], bf16)
        for l in range(layer_num - 1)
    ]

    with tile.TileContext(nc) as tc:
        with (
            tc.tile_pool(name="wpool", bufs=1) as wpool,
            tc.tile_pool(name="cpool", bufs=1) as cpool,
            tc.tile_pool(name="hpool", bufs=1) as hpool,
            tc.tile_pool(name="respool", bufs=2) as respool,
            tc.tile_pool(name="qkpool", bufs=4) as qkpool,
            tc.tile_pool(name="vpool", bufs=1) as vpool,
            tc.tile_pool(name="epool", bufs=2) as epool,
            tc.tile_pool(name="opool", bufs=1) as opool,
            tc.tile_pool(name="oppool", bufs=1) as oppool,
            tc.tile_pool(name="h1pool", bufs=1) as h1pool,
            tc.tile_pool(name="strm", bufs=2) as strm,
            tc.tile_pool(name="small", bufs=1) as small,
            tc.tile_pool(name="psA", bufs=2, space="PSUM") as psA,
            tc.tile_pool(name="psB", bufs=2, space="PSUM") as psB,
        ):
            # ------------- load weights/consts -------------
            wq8t = wpool.tile([P, FS, 2 * P], bf16)
            wkt = wpool.tile([P, FS, 2 * P], bf16)
            wvt = wpool.tile([P, FS, 2 * P], bf16)
            wot = wpool.tile([P, FS, DM], bf16)
            w1t = wpool.tile([P, FS, DFF], bf16)
            w2t = wpool.tile([P, DS2, DM], bf16)
            # order by first use: wv + hT feed P1 immediately; wq/wk feed P2;
            # wo only feeds P4.  The ~16 tiny const DMAs go on the (otherwise
            # idle at startup) scalar/gpsimd queues — each dma issue costs
            # ~600ns, and serializing them on sync ahead of hT delayed the
            # first matmul by ~13us.  (dma_start is only legal on sync,
            # gpsimd, and scalar.)
            nc.sync.dma_start(wvt, wv[:])
            hT0 = hpool.tile([P, FS, S], bf16, tag="hT")
            for sf in range(FS):
                nc.sync.dma_start(hT0[:, sf], xT[:][:, sf])

            bqt = cpool.tile([P, 2], f32)
            bkt = cpool.tile([P, 2], f32)
            bvt = cpool.tile([P, 2 * P], f32)
            bot = cpool.tile([P, FS], f32)
            b1t = cpool.tile([P, DS2], f32)
            b2t = cpool.tile([P, FS], f32)
            g1t = cpool.tile([P, FS], f32)
            be1t = cpool.tile([P, FS], f32)
            g2t = cpool.tile([P, FS], f32)
            be2t = cpool.tile([P, FS], f32)
            oK1t = cpool.tile([1, P], f32r)
            idt = cpool.tile([P, P], f32r)
            rm128t = cpool.tile([P, 1], f32r)
            borowt = cpool.tile([1, DM], bf16)
            b2rowt = cpool.tile([1, DM], bf16)
            onestokt = cpool.tile([1, TOK], bf16)
            for n, (t, src) in enumerate(
                    ((bqt, biasq), (bkt, biask), (bvt, bvb), (bot, bo_g),
                     (b1t, b1_g), (b2t, b2_g), (g1t, g1_g), (be1t, be1_g),
                     (g2t, g2_g), (be2t, be2_g),
                     (oK1t, onesK1), (idt, identd),
                     (rm128t, rm128d), (borowt, borow_d), (b2rowt, b2row_d),
                     (onestokt, onestok_d))):
                eng = nc.scalar if n % 2 == 0 else nc.gpsimd
                eng.dma_start(t, src[:])

            pid = nc.gpsimd.partition_id()
            ppart = (pid + 1) % 2        # partner's slot within my pair

            res_prev = None
            for l in range(layer_num):
                last = l == layer_num - 1
                # ------------- token sources (core-relative order) ----------
                # local tokens 0..TOK-1, partner tokens TOK..S-1
                if l == 0:
                    hT = hT0
                    for t, src in ((wq8t, wq8), (wkt, wk), (wot, wo)):
                        nc.sync.dma_start(t, src[:])
                    res0t = respool.tile([P, FS, TOK], f32r, tag="res")
                    nc.sync.dma_start(res0t, res0[:])

                    def tok128(sf, jb):
                        return hT[:, sf, jb * P:(jb + 1) * P]

                    def tok512(sf, half):
                        return hT[:, sf, half * TOK:(half + 1) * TOK]

                    def res_src(fc):
                        return res0t[:, fc, :]
                else:
                    hprev_bf = hb_prev
                    hres = res_prev
                    # reuse the (dead after layer 0) hT slot for hrem
                    hrem = hpool.tile([P, FS, TOK], bf16, tag="hT")
                    hsrc = h_out[l - 1][:]
                    for sf in range(FS):
                        nc.gpsimd.dma_start(
                            hrem[:, sf, :],
                            hsrc[bass.ts(ppart, 1)][0].rearrange(
                                "sf p t -> p sf t")[:, sf],
                        )

                    def tok128(sf, jb):
                        if jb < 4:
                            return hprev_bf[:, sf, jb * P:(jb + 1) * P]
                        return hrem[:, sf, (jb - 4) * P:(jb - 3) * P]

                    def tok512(sf, half):
                        return (hprev_bf if half == 0 else hrem)[:, sf, :]

                    def res_src(fc):
                        return hres[:, fc, :]

                # ---------------- P1/P2/P3: v projection, q/k, attention ------
                # (ones column appended per head so the Ev matmul also
                # accumulates Z in PSUM row 64)
                v_t = vpool.tile([P, JBN, HPC, 65], bf16, tag="v")
                nc.gpsimd.memset(v_t[:, :, :, 64:65], 1.0)
                oT_all = opool.tile([P, 2, S], bf16, tag="obuf")
                opart = oppool.tile([P, 2, TOK], bf16, tag="opart")
                pending_z = []

                def flush_z():
                    while pending_z:
                        pending_z.pop(0)()

                def v_block(jb):
                    psv = psB.tile([P, 2 * P], f32, tag="psB",
                                   name=f"psv_{l}_{jb}")
                    for sf in range(FS):
                        nc.tensor.matmul(
                            psv, tok128(sf, jb), wvt[:, sf, :],
                            start=(sf == 0), stop=(sf == FS - 1),
                        )
                    for q4 in range(HPC):
                        nc.vector.tensor_tensor(
                            v_t[:, jb, q4, 0:64], psv[:, q4 * 64:(q4 + 1) * 64],
                            bvt[:, q4 * 64:(q4 + 1) * 64], ALU.add)

                def qk_mm(ps, w_t, pr, tc2):
                    for sf in range(FS):
                        nc.tensor.matmul(
                            ps[:, tc2 * 512:(tc2 + 1) * 512],
                            w_t[:, sf, pr * P:(pr + 1) * P],
                            tok512(sf, tc2),
                            start=(sf == 0), stop=(sf == FS - 1),
                        )

                def qk_copies(pr, which, ps, b_t, rsrc, pair_tiles):
                    for hh in range(2):
                        til = qkpool.tile([66, S], bf16, tag="qk",
                                          name=f"qk_{l}_{pr}_{which}_{hh}")
                        nc.vector.tensor_scalar_add(
                            til[0:64, :],
                            ps[hh * 64:(hh + 1) * 64, :],
                            b_t[hh * 64:(hh + 1) * 64, pr:pr + 1])
                        nc.sync.dma_start(til[64:66, :], rsrc[:])
                        pair_tiles[(which, hh)] = til

                def attention_head(pr, hh, qt, kt, ship=False):
                    # software pipeline: issue logits(jb) on the PE before
                    # Ev(jb-1), so the PE never stalls on the scalar engine's
                    # exp(jb-1); the z-normalization PE/scale phase of the
                    # PREVIOUS head is emitted mid-loop (jb==3), by which time
                    # its eagerly-issued vector phase has long completed, so
                    # the tiny broadcast matmul never stalls the PE queue
                    hl = pr * 2 + hh
                    oT_ps = psB.tile([65, S], f32, tag="psB",
                                     name=f"oT_ps_{l}_{hl}")
                    Es = []
                    for jb in range(JBN + 1):
                        if jb < JBN:
                            l_ps = psA.tile([P, S], f32, tag="psA",
                                            name=f"l_ps_{l}_{hl}_{jb}")
                            for ic in range(2):
                                nc.tensor.matmul(
                                    l_ps[:, ic * 512:(ic + 1) * 512],
                                    qt[:, jb * P:(jb + 1) * P],
                                    kt[:, ic * 512:(ic + 1) * 512],
                                    start=True, stop=True,
                                )
                            if structured:
                                esrc = l_ps
                            else:
                                lm = strm.tile([P, S], f32, tag="lm")
                                ng = strm.tile([P, S], f32, tag="ng")
                                nc.sync.dma_start(ng, negm[:][:, jb])
                                nc.vector.tensor_tensor(lm, l_ps, ng, ALU.add)
                                esrc = lm
                            E = epool.tile([P, S], bf16, tag="E",
                                           name=f"E_{l}_{hl}_{jb}")
                            nc.scalar.activation(E, esrc, FT.Exp, bias=-EXPB)
                            Es.append(E)
                        if jb == 3:
                            flush_z()
                        if jb >= 1:
                            pj = jb - 1
                            for ic in range(2):
                                nc.tensor.matmul(
                                    oT_ps[:, ic * 512:(ic + 1) * 512],
                                    v_t[:, pj, hl, 0:65],
                                    Es[pj][:, ic * 512:(ic + 1) * 512],
                                    start=(pj == 0), stop=(pj == JBN - 1),
                                )

                    # phase 1 (vector, eager): Z = sum_i (row 64 of oT_ps)
                    z1 = small.tile([1, 1], f32r, tag="z1",
                                    name=f"z1_{l}_{hl}")
                    # f32r is bit-identical to f32; the lint only keys on
                    # the dtype tag
                    with nc.allow_low_precision(reason="f32r == f32 bits"):
                        nc.vector.reduce_sum(z1, oT_ps[64:65, :],
                                             axis=mybir.AxisListType.X)
                    zrow8 = small.tile([1, 8], f32r, tag="zrow8",
                                       name=f"zrow8_{l}_{hl}")
                    nc.vector.tensor_copy(zrow8, z1.to_broadcast((1, 8)))

                    def zchain(oT_ps=oT_ps, zrow8=zrow8, pr=pr, hh=hh, hl=hl,
                               ship=ship):
                        # phase 2 (deferred): partition-broadcast z via a K=1
                        # matmul (N=8: the ISA rejects N=1 f32r moving
                        # operands), then scale o by nz/Z
                        zb_ps = psA.tile([64, 8], f32, tag="psA",
                                         name=f"zb_ps_{l}_{hl}")
                        nc.tensor.matmul(zb_ps, oK1t[:, 0:64], zrow8,
                                         start=True, stop=True)
                        zz = small.tile([64, 1], f32, tag="zz",
                                        name=f"zz_{l}_{hl}")
                        nc.vector.reciprocal(zz, zb_ps[:, 0:1])
                        nc.vector.tensor_scalar_mul(zz, zz, float(nz))
                        nc.vector.tensor_tensor(
                            oT_all[hh * 64:hh * 64 + 64, pr, :],
                            oT_ps[0:64, :], zz.to_broadcast((64, S)), ALU.mult)
                        if ship:
                            o_exchange(pr)
                    pending_z.append(zchain)

                def o_exchange(pr):
                    # ship my heads' o on the PARTNER's token block (pairwise);
                    # gpsimd queue so it isn't stuck behind stalled sync DMAs
                    nc.gpsimd.dma_start(o_in[l][pr][:],
                                        oT_all[:, pr, TOK:2 * TOK])
                    nc.gpsimd.collective_compute(
                        "AllGather", ALU.bypass,
                        replica_groups=PAIRS,
                        ins=[o_in[l][pr][:]], outs=[o_out[l][pr][:]],
                    )
                    osrc = o_out[l][pr][:]
                    nc.gpsimd.dma_start(opart[:, pr, :],
                                        osrc[bass.ts(ppart, 1)][0])

                specs = (("q", wq8t, bqt, qrow), ("k", wkt, bkt, krow))
                pair0, pair1 = {}, {}
                if l > 0:
                    # interleave local/remote so the h exchange hides behind
                    # local-token compute
                    for jb in range(4):
                        v_block(jb)
                    pss = {}
                    for which, w_t, b_t, rsrc in specs:
                        ps = psA.tile([P, S], f32, tag="psA",
                                      name=f"qk_ps_{l}_0_{which}")
                        qk_mm(ps, w_t, 0, 0)
                        pss[which] = ps
                    for jb in range(4, JBN):
                        v_block(jb)
                    for which, w_t, b_t, rsrc in specs:
                        qk_mm(pss[which], w_t, 0, 1)
                        qk_copies(0, which, pss[which], b_t, rsrc, pair0)
                else:
                    for jb in range(JBN):
                        v_block(jb)
                    # deferred big weight loads: issued after P1 so the layer-0
                    # projections aren't queued behind 4MB of FFN weights
                    for sf in range(FS):
                        nc.sync.dma_start(w1t[:, sf], w1[:][:, sf])
                    for s2 in range(0, DS2, 4):
                        nc.sync.dma_start(w2t[:, s2:s2 + 4], w2[:][:, s2:s2 + 4])
                    for which, w_t, b_t, rsrc in specs:
                        ps = psA.tile([P, S], f32, tag="psA",
                                      name=f"qk_ps_{l}_0_{which}")
                        qk_mm(ps, w_t, 0, 0)
                        qk_mm(ps, w_t, 0, 1)
                        qk_copies(0, which, ps, b_t, rsrc, pair0)

                attention_head(0, 0, pair0[("q", 0)], pair0[("k", 0)])
                attention_head(0, 1, pair0[("q", 1)], pair0[("k", 1)], ship=True)
                # pr=0's last z-chain + o exchange are deferred into pr=1's
                # first attention head, so the PE rolls straight into pr=1's
                # q/k projections
                for which, w_t, b_t, rsrc in specs:
                    ps = psA.tile([P, S], f32, tag="psA",
                                  name=f"qk_ps_{l}_1_{which}")
                    qk_mm(ps, w_t, 1, 0)
                    qk_mm(ps, w_t, 1, 1)
                    qk_copies(1, which, ps, b_t, rsrc, pair1)
                attention_head(1, 0, pair1[("q", 0)], pair1[("k", 0)])
                attention_head(1, 1, pair1[("q", 1)], pair1[("k", 1)], ship=True)
                flush_z()

                # ---------------- P4: attn out + residual + LN1 ---------------
                # feature subtiles of o: 0,1 = my heads (pr 0,1; local in
                # oT_all), 2,3 = partner heads (from opart); wo rows are
                # permuted to match on the host.
                h1T = h1pool.tile([P, FS, TOK], f32r, tag="h1")
                for fc in range(FS):
                    ps = psA.tile([P, TOK], f32, tag="psA")
                    nc.tensor.matmul(ps, borowt[:, fc * P:(fc + 1) * P], onestokt,
                                     start=True, stop=False)
                    for pr in range(2):
                        nc.tensor.matmul(
                            ps, wot[:, pr, fc * P:(fc + 1) * P],
                            oT_all[:, pr, 0:TOK],
                            start=False, stop=False,
                        )
                    for pr in range(2):
                        nc.tensor.matmul(
                            ps, wot[:, 2 + pr, fc * P:(fc + 1) * P],
                            opart[:, pr, :],
                            start=False, stop=(pr == 1),
                        )
                    nc.vector.tensor_tensor(h1T[:, fc, :], ps, res_src(fc), ALU.add)
                h1nT = h1pool.tile([P, FS, TOK], f32r, tag="h1n")
                h1nB = h1pool.tile([P, FS, TOK], bf16, tag="h1b")
                # FFN accumulators are allocated early so their bias-init
                # matmuls can fill the PE bubble in LN1's stats->mb chain
                f2a = psA.tile([P, S], f32, tag="psA")
                f2b = psA.tile([P, S], f32, tag="psA")

                def ffn_bias_init():
                    for fc in range(FS):
                        dst = f2a if fc < 2 else f2b
                        nc.tensor.matmul(
                            dst[:, (fc % 2) * TOK:(fc % 2 + 1) * TOK],
                            b2rowt[:, fc * P:(fc + 1) * P], onestokt,
                            start=True, stop=False)

                _layernorm(nc, psA, psB, strm, small, h1T, h1nT, rm128t, oK1t,
                           g1t, be1t, xout_bf=h1nB, pe_filler=ffn_bias_init)

                # ---------------- P5: FFN + residual + LN2 --------------------
                # software-pipelined: p1(s2) issues before p2(s2-1) so the PE
                # never stalls on the vector engine's relu
                a_ts = []
                for s2 in range(DS2 + 1):
                    if s2 < DS2:
                        p1 = psB.tile([P, TOK], f32, tag="psB")
                        for sf in range(FS):
                            nc.tensor.matmul(
                                p1, w1t[:, sf, s2 * P:(s2 + 1) * P],
                                h1nB[:, sf, :],
                                start=(sf == 0), stop=(sf == FS - 1),
                            )
                    if s2 >= 1:
                        p = s2 - 1
                        for fc in range(FS):
                            dst = f2a if fc < 2 else f2b
                            nc.tensor.matmul(
                                dst[:, (fc % 2) * TOK:(fc % 2 + 1) * TOK],
                                w2t[:, p, fc * P:(fc + 1) * P], a_ts[p],
                                start=False, stop=(p == DS2 - 1),
                            )
                    if s2 < DS2:
                        a_t = strm.tile([P, TOK], bf16, tag="aT", bufs=3)
                        nc.vector.tensor_scalar(a_t, p1, b1t[:, s2:s2 + 1], 0.0,
                                                ALU.add, ALU.max)
                        a_ts.append(a_t)
                h2T = respool.tile([P, FS, TOK], f32r, tag="res")
                for fc in range(FS):
                    src_ps = f2a if fc < 2 else f2b
                    sl = src_ps[:, (fc % 2) * TOK:(fc % 2 + 1) * TOK]
                    nc.vector.tensor_tensor(h2T[:, fc, :], sl, h1nT[:, fc, :], ALU.add)
                h2B = None
                if not last:
                    h2B = respool.tile([P, FS, TOK], bf16, tag="resb",
                                       name="h2B")
                _layernorm(nc, psA, psB, strm, small, h2T, h2T, rm128t, oK1t,
                           g2t, be2t, xout_bf=h2B)
                res_prev = h2T
                hb_prev = h2B

                if not last:
                    hdst = h_in[l][:]
                    for sf in range(FS):
                        nc.sync.dma_start(hdst[sf], h2B[:, sf, :])
                    nc.gpsimd.collective_compute(
                        "AllGather", ALU.bypass,
                        replica_groups=PAIRS,
                        ins=[h_in[l][:]], outs=[h_out[l][:]],
                    )
                else:
                    out_sb = hpool.tile([P, FS, DM], f32, tag="hT")
                    for sf in range(FS):
                        for tc4 in range(FS):
                            tp = psB.tile([P, P], f32r, tag="psB")
                            nc.tensor.transpose(
                                tp, h2T[:, sf, tc4 * P:(tc4 + 1) * P], idt)
                            nc.vector.tensor_copy(out_sb[:, tc4, sf * P:(sf + 1) * P],
                                                  tp)
                    nc.sync.dma_start(
                        out[:].rearrange("(tb p) f -> p tb f", p=P), out_sb)

    nc.compile()
    return nc


def _layernorm(nc, psA, psB, strm, small, xin, xout, rm128t, oK1t, gt, bt,
               xout_bf=None, pe_filler=None):
    """Feature-major LayerNorm: xin/xout [P, FS, TOK] f32r.  Stats via
    (1/DM)-matmul over partitions (mean and E[x^2] directly); squares on ACT;
    rstd = exp(-0.5*ln(var+eps)) with eps folded into the Ln bias and -0.5
    into the Exp scale; normalize written in place.  Optionally also emits a
    bf16 shadow copy (matmul-operand precision) via the scalar engine."""
    stats = psB.tile([1, 2 * TOK], f32, tag="psB")
    for sf in range(FS):
        nc.tensor.matmul(stats[:, 0:TOK], rm128t, xin[:, sf, :],
                         start=(sf == 0), stop=(sf == FS - 1))
    for sf in range(FS):
        sq = strm.tile([P, TOK], f32r, tag="sq")
        nc.scalar.activation(sq, xin[:, sf, :], FT.Square)
        nc.tensor.matmul(stats[:, TOK:2 * TOK], rm128t, sq,
                         start=(sf == 0), stop=(sf == FS - 1))
    if pe_filler is not None:
        # independent PE work to cover the vector/ACT stats chain below
        pe_filler()
    mrs = small.tile([1, 2 * TOK], f32r, tag="mrs")
    nc.vector.tensor_copy(mrs[:, 0:TOK], stats[:, 0:TOK])
    msq = small.tile([1, TOK], f32, tag="msq")
    nc.vector.tensor_tensor(msq, mrs[:, 0:TOK], mrs[:, 0:TOK], ALU.mult)
    vtmp = small.tile([1, TOK], f32, tag="vtmp")
    nc.vector.tensor_tensor(vtmp, stats[:, TOK:2 * TOK], msq, ALU.subtract)
    nc.scalar.activation(vtmp, vtmp, FT.Ln, bias=EPS)
    nc.scalar.activation(mrs[:, TOK:2 * TOK], vtmp, FT.Exp, scale=-0.5)
    mb = psB.tile([P, 2 * TOK], f32, tag="psB")
    for half in range(2):
        nc.tensor.matmul(mb[:, half * TOK:(half + 1) * TOK], oK1t,
                         mrs[:, half * TOK:(half + 1) * TOK],
                         start=True, stop=True)
    for sf in range(FS):
        nc.vector.tensor_tensor(xout[:, sf, :], xin[:, sf, :], mb[:, 0:TOK],
                                ALU.subtract)
        nc.vector.tensor_tensor(xout[:, sf, :], xout[:, sf, :],
                                mb[:, TOK:2 * TOK], ALU.mult)
        nc.vector.tensor_scalar(xout[:, sf, :], xout[:, sf, :],
                                gt[:, sf:sf + 1], bt[:, sf:sf + 1],
                                ALU.mult, ALU.add)
        if xout_bf is not None:
            nc.scalar.activation(xout_bf[:, sf, :], xout[:, sf, :], FT.Copy)


# ---------------------------------------------------------------------------
# Host side
# ---------------------------------------------------------------------------

def _feature_major(x2d, dt=BF):
    """[T, F] -> [P, F//P, T] layout array (contiguous)."""
    t, f = x2d.shape
    return np.ascontiguousarray(
        x2d.T.reshape(f // P, P, t).transpose(1, 0, 2)).astype(dt)


def _lhsT_layout(w):
    """[K, M] -> [P, K//P, M] (bf16)."""
    k, m = w.shape
    return np.ascontiguousarray(
        w.reshape(k // P, P, m).transpose(1, 0, 2)).astype(BF)


def _per_partition(vec):
    """[F] -> [P, F//P] (partition-major blocks of 128)."""
    f = vec.shape[0]
    return np.ascontiguousarray(vec.reshape(f // P, P).T).astype(np.float32)


_PROGRAM_CACHE = {}

# Optional profiling knobs (used by test.py; harmless defaults for grading).
TRACE = False
TRACE_KW = {}
LAST_RESULTS = None


def kernel(**inputs):
    x = np.asarray(inputs["x"], np.float32)
    mask = np.asarray(inputs["mask"], np.float32)
    protok = np.asarray(inputs["protok"])
    layer_num = int(np.asarray(inputs["layer_num"]))
    if layer_num <= 0:
        return x.copy()

    nz = float(np.count_nonzero(np.asarray(protok)[0]))

    pad = np.ascontiguousarray(np.einsum("bii->bi", mask))
    structured = bool(
        np.all((pad == 0) | (pad == 1))
        and np.array_equal(mask, np.maximum(pad[:, :, None], pad[:, None, :]))
    )

    key = (layer_num, nz, structured)
    if key not in _PROGRAM_CACHE:
        _PROGRAM_CACHE[key] = build_program(layer_num, nz, structured)
    nc = _PROGRAM_CACHE[key]
    in_maps = make_in_maps(inputs, x, mask, pad, structured)
    kw = dict(trace=True, **TRACE_KW) if TRACE else {}
    res = run_bass_kernel_spmd(nc, in_maps, core_ids=list(range(NC)), **kw)
    globals()["LAST_RESULTS"] = res
    outp = np.empty((B, S, DM), np.float32)
    for c in range(NC):
        b, g = c // 2, c % 2
        outp[b, g * TOK:(g + 1) * TOK] = res.results[c]["out"]
    return outp


def make_in_maps(inputs, x, mask, pad, structured):
    wq8 = inputs["wq"] / 8.0
    bq8 = np.asarray(inputs["bq"], np.float32) / 8.0
    wo_np = np.asarray(inputs["wo"], np.float32)
    ident = np.eye(P, dtype=np.float32)
    in_maps = []
    for c in range(NC):
        b, g = c // 2, c % 2
        hcols = slice(g * 2 * P, (g + 1) * 2 * P)
        # core-relative token order: own block first, partner block second
        idx = np.r_[g * TOK:(g + 1) * TOK, (1 - g) * TOK:(2 - g) * TOK]
        # wo rows in core-relative head order: my heads' features first
        mine = slice(g * 2 * P, (g + 1) * 2 * P)
        theirs = slice((1 - g) * 2 * P, (2 - g) * 2 * P)
        wo_p = np.concatenate([wo_np[mine], wo_np[theirs]], axis=0)
        m = {
            "xT": _feature_major(x[b][idx]).reshape(P, FS, S),
            "res0": _feature_major(x[b, g * TOK:(g + 1) * TOK],
                                   np.float32).reshape(P, FS, TOK),
            "wq8": _lhsT_layout(np.asarray(wq8, np.float32)[:, hcols]),
            "wk": _lhsT_layout(np.asarray(inputs["wk"], np.float32)[:, hcols]),
            "wv": _lhsT_layout(np.asarray(inputs["wv"], np.float32)[:, hcols]),
            "wo": _lhsT_layout(wo_p),
            "w1": _lhsT_layout(np.asarray(inputs["w1"], np.float32)),
            "w2": _lhsT_layout(np.asarray(inputs["w2"], np.float32)),
            "biasq": _per_partition(bq8[hcols]),
            "biask": _per_partition(np.asarray(inputs["bk"], np.float32)[hcols]),
            "bvb": np.broadcast_to(
                np.asarray(inputs["bv"], np.float32)[hcols], (P, 2 * P)).copy(),
            "bo_g": _per_partition(np.asarray(inputs["bo"], np.float32)),
            "b1_g": _per_partition(np.asarray(inputs["b1"], np.float32)),
            "b2_g": _per_partition(np.asarray(inputs["b2"], np.float32)),
            "g1_g": _per_partition(np.asarray(inputs["ln1_g"], np.float32)),
            "be1_g": _per_partition(np.asarray(inputs["ln1_b"], np.float32)),
            "g2_g": _per_partition(np.asarray(inputs["ln2_g"], np.float32)),
            "be2_g": _per_partition(np.asarray(inputs["ln2_b"], np.float32)),
            "onesK1": np.ones((1, P), np.float32),
            "rm128d": np.full((P, 1), 1.0 / DM, np.float32),
            "borow_d": np.asarray(inputs["bo"], np.float32).reshape(1, DM).astype(BF),
            "b2row_d": np.asarray(inputs["b2"], np.float32).reshape(1, DM).astype(BF),
            "onestok_d": np.ones((1, TOK), BF),
            "identd": ident,
        }
        padp = pad[b][idx]
        if structured:
            m["qrow"] = np.stack([-1e9 * padp, np.ones(S, np.float32)]).astype(BF)
            m["krow"] = np.stack([np.ones(S, np.float32), -1e9 * padp]).astype(BF)
        else:
            m["qrow"] = np.zeros((2, S), BF)
            m["krow"] = np.zeros((2, S), BF)
            m["negm"] = np.ascontiguousarray(
                (-1e9 * mask[b][np.ix_(idx, idx)]).reshape(
                    JBN, P, S).transpose(1, 0, 2))
        in_maps.append(m)
    return in_maps
